# revision 18
# baseline (speedup 1.0000x reference)
"""Trainium2 Bass kernel for nn_DWTExtractor: 2-level Haar DWT + bilinear 2x upsample.

Input  x: (32, 1, 1024, 1024) fp32
Output y: (32, 6, 512, 512) fp32 = [cH1, cV1, cD1, cH2u, cV2u, cD2u]

Sharding: pure batch data-parallel, 4 images per core across 8 cores.

v3: fp16 datapath to halve HBM traffic (the kernel is memory-bound).
  - Host converts x to fp16; device reads fp16, writes fp16 outputs; host
    converts back to fp32. Max rel err ~6e-4 (validated in sim_v2.py).
  - L1/L2 Haar row-pairing as fp16 matmuls (+-0.5 weights), PSUM fp32 held
    as [P,1024]/[P,512] tiles.
  - Engines may read at most ONE operand from PSUM, so ACT makes one strided
    fp32 copy of the even W-columns per PSUM tile; DVE/GPSIMD then combine
    it with the strided odd columns read directly from PSUM:
      cH1 = Se - So (DVE), cA1 = Se + So (GPSIMD, fp16 for L2 rhs),
      cV1 = De + Do, cD1 = De - Do (DVE); same pattern for the L2 bands.
  - L2 uses explicit cA1 chunks -> 2 matmuls per tile.
  - W-upsample folds the x3 into scalar_tensor_tensor (split DVE/GPSIMD).
  - H-upsample in fp16, block-major with a "halo" row swap between the two
    wu tiles (SBUF->SBUF DMA) so the cross-block U1b/U2b correction matmuls
    disappear: 12 matmuls instead of 18 per image.
  - stage_b is split so the halo DMA latency hides behind the next image's
    L1 matmuls; H-up PSUM evacuation is spread over ACT/DVE/GPSIMD.
"""

import numpy as np

import concourse.bass as bass
import concourse.tile as tile
import concourse.mybir as mybir
from concourse import bacc, bass_utils

F32 = mybir.dt.float32
F16 = mybir.dt.float16
AL = mybir.AluOpType

B, H, W = 32, 1024, 1024
NCORES = 8
IMG = B // NCORES  # images per core
HL, WL = H // 2, W // 2  # 512, 512 (level-1 band size)
H2, W2 = H // 4, W // 4  # 256, 256 (level-2 band size)
P = 128


def _build_w16() -> np.ndarray:
    """(128, 8*128) fp16: PS_lo|PS_hi|PD_lo|PD_hi|U0|U1p|U2p|U3.

    U1p/U2p carry the cross-block bilinear taps in rows 0/127; the matching
    rhs rows are swapped between the wu tiles at runtime (halo DMA).
    """
    ps_lo = np.zeros((P, P), np.float16)
    ps_hi = np.zeros((P, P), np.float16)
    pd_lo = np.zeros((P, P), np.float16)
    pd_hi = np.zeros((P, P), np.float16)
    for i in range(64):
        ps_lo[2 * i, i] = 0.5
        ps_lo[2 * i + 1, i] = 0.5
        ps_hi[2 * i, 64 + i] = 0.5
        ps_hi[2 * i + 1, 64 + i] = 0.5
        pd_lo[2 * i, i] = 0.5
        pd_lo[2 * i + 1, i] = -0.5
        pd_hi[2 * i, 64 + i] = 0.5
        pd_hi[2 * i + 1, 64 + i] = -0.5

    u_full = np.zeros((H2, HL), np.float32)
    for m in range(HL):
        k = m // 2
        taps = [(k, 0.75), (k - 1, 0.25)] if m % 2 == 0 else [(k, 0.75), (k + 1, 0.25)]
        for src, wgt in taps:
            u_full[min(max(src, 0), H2 - 1), m] += wgt
    u_full *= 0.25
    u0 = u_full[0:128, 0:128].astype(np.float16)
    u1p = u_full[0:128, 128:256].astype(np.float16)
    u1p[0, :] = u_full[128, 128:256].astype(np.float16)  # halo tap row
    u2p = u_full[128:256, 256:384].astype(np.float16)
    u2p[127, :] = u_full[127, 256:384].astype(np.float16)  # halo tap row
    u3 = u_full[128:256, 384:512].astype(np.float16)

    return np.concatenate([ps_lo, ps_hi, pd_lo, pd_hi, u0, u1p, u2p, u3], axis=1)


def build_nc() -> "bacc.Bacc":
    nc = bacc.Bacc(
        "TRN2", target_bir_lowering=False, debug=False, num_devices=NCORES,
        name="dwt_extractor",
    )
    x_d = nc.dram_tensor("xc", [IMG, H, W], F16, kind="ExternalInput")
    w16_d = nc.dram_tensor("w16", [P, 8 * P], F16, kind="ExternalInput")
    y_d = nc.dram_tensor("yc", [IMG, 6, HL, WL], F16, kind="ExternalOutput")

    with tile.TileContext(nc) as tc:
        with (
            tc.tile_pool(name="consts", bufs=1) as cpool,
            tc.tile_pool(name="xin", bufs=8) as xpool,
            tc.tile_pool(name="ev", bufs=3) as evpool,
            tc.tile_pool(name="ca", bufs=6) as capool,
            tc.tile_pool(name="stg", bufs=2) as stgpool,
            tc.tile_pool(name="b3", bufs=3) as b3pool,
            tc.tile_pool(name="wup", bufs=4) as wuppool,
            tc.tile_pool(name="stg2", bufs=2) as stg2pool,
            tc.tile_pool(name="psS", bufs=1, space="PSUM") as psS,
            tc.tile_pool(name="psD", bufs=1, space="PSUM") as psD,
            tc.tile_pool(name="psL2", bufs=1, space="PSUM") as psL2,
            tc.tile_pool(name="psUp", bufs=2, space="PSUM") as psUp,
        ):
            w16 = cpool.tile([P, 8 * P], F16)
            nc.sync.dma_start(w16[:], w16_d[:])
            blk = lambda i: w16[:, i * P : (i + 1) * P]
            PS_lo, PS_hi, PD_lo, PD_hi = blk(0), blk(1), blk(2), blk(3)
            U0, U1p, U2p, U3 = blk(4), blk(5), blk(6), blk(7)

            def stage_a(b):
                """L1 chunks + L2 + W-upsample for image b; returns wup3s."""
                ca = []
                stgL1 = []
                for u in range(4):
                    xu = xpool.tile([P, 2048], F16, tag="x")
                    src = x_d[b, 256 * u : 256 * (u + 1), :]
                    nc.sync.dma_start(
                        xu[:].rearrange("p (t w) -> p t w", t=2),
                        src.rearrange("(t p) w -> p t w", t=2),
                    )
                    if u == 0:
                        stgH1 = stgpool.tile([P, 2048], F16, tag="sH1")
                        stgV1 = stgpool.tile([P, 2048], F16, tag="sV1")
                        stgD1 = stgpool.tile([P, 2048], F16, tag="sD1")
                        stgL1 = [stgH1, stgV1, stgD1]
                    o512 = 512 * u
                    sS = psS.tile([P, 1024], F32, tag="S")
                    for h in range(2):
                        o = 512 * h
                        nc.tensor.matmul(
                            sS[:, o : o + 512], PS_lo, xu[:, o : o + 512],
                            start=True, stop=False,
                        )
                        nc.tensor.matmul(
                            sS[:, o : o + 512], PS_hi, xu[:, 1024 + o : 1536 + o],
                            start=False, stop=True,
                        )
                    se = evpool.tile([P, 512], F16, tag="se")
                    nc.scalar.copy(se[:], sS[:, 0:1024:2])
                    cu = capool.tile([P, 512], F16, tag="ca")
                    nc.vector.tensor_tensor(
                        stgL1[0][:, o512 : o512 + 512],
                        se[:], sS[:, 1:1024:2], AL.subtract,
                    )  # cH1
                    nc.vector.tensor_tensor(
                        cu[:], se[:], sS[:, 1:1024:2], AL.add
                    )  # cA1
                    ca.append(cu)

                    sD = psD.tile([P, 1024], F32, tag="D")
                    for h in range(2):
                        o = 512 * h
                        nc.tensor.matmul(
                            sD[:, o : o + 512], PD_lo, xu[:, o : o + 512],
                            start=True, stop=False,
                        )
                        nc.tensor.matmul(
                            sD[:, o : o + 512], PD_hi, xu[:, 1024 + o : 1536 + o],
                            start=False, stop=True,
                        )
                    de = evpool.tile([P, 512], F16, tag="de")
                    nc.scalar.copy(de[:], sD[:, 0:1024:2])
                    nc.vector.tensor_tensor(
                        stgL1[1][:, o512 : o512 + 512],
                        de[:], sD[:, 1:1024:2], AL.add,
                    )  # cV1
                    nc.vector.tensor_tensor(
                        stgL1[2][:, o512 : o512 + 512],
                        de[:], sD[:, 1:1024:2], AL.subtract,
                    )  # cD1

                # level 2 + W-upsample; wup3s[v] = (128, 3*512) fp16
                wup3s = [None, None]
                for v in range(2):
                    s2 = psL2.tile([P, 512], F32, tag="s2")
                    nc.tensor.matmul(s2[:], PS_lo, ca[2 * v][:], start=True, stop=False)
                    nc.tensor.matmul(s2[:], PS_hi, ca[2 * v + 1][:], start=False, stop=True)
                    d2 = psL2.tile([P, 512], F32, tag="d2")
                    nc.tensor.matmul(d2[:], PD_lo, ca[2 * v][:], start=True, stop=False)
                    nc.tensor.matmul(d2[:], PD_hi, ca[2 * v + 1][:], start=False, stop=True)

                    s2e = evpool.tile([P, 256], F16, tag="s2e")
                    nc.scalar.copy(s2e[:], s2[:, 0:512:2])
                    d2e = evpool.tile([P, 256], F16, tag="d2e")
                    nc.scalar.copy(d2e[:], d2[:, 0:512:2])

                    b3 = b3pool.tile([P, 768], F16, tag="b3")
                    nc.vector.tensor_tensor(
                        b3[:, 0:256], s2e[:], s2[:, 1:512:2], AL.subtract
                    )  # cH2
                    nc.vector.tensor_tensor(
                        b3[:, 256:512], d2e[:], d2[:, 1:512:2], AL.add
                    )  # cV2
                    nc.vector.tensor_tensor(
                        b3[:, 512:768], d2e[:], d2[:, 1:512:2], AL.subtract
                    )  # cD2

                    # W-upsample: wu[2j] = 3*b[j] + b[j-1]; wu[2j+1] = 3*b[j] + b[j+1]
                    # GPSIMD has no scalar_tensor_tensor (and its fp16
                    # tensor_scalar is a very slow emulation), so stage
                    # t3 = 3*b3 on DVE and keep the adds on GPSIMD.
                    t3 = b3pool.tile([P, 768], F16, tag="t3")
                    nc.vector.tensor_scalar_mul(t3[:], b3[:], 3.0)
                    wu = wuppool.tile([P, 1536], F16, tag="wup")
                    wu_r = wu[:].rearrange("p (b w) -> p b w", b=3)
                    b3_r = b3[:].rearrange("p (b w) -> p b w", b=3)
                    t3_r = t3[:].rearrange("p (b w) -> p b w", b=3)
                    nc.gpsimd.tensor_tensor(
                        wu_r[:, :, 2:512:2], t3_r[:, :, 1:256],
                        b3_r[:, :, 0:255], AL.add,
                    )
                    nc.gpsimd.tensor_tensor(
                        wu_r[:, :, 1:511:2], t3_r[:, :, 0:255],
                        b3_r[:, :, 1:256], AL.add,
                    )
                    nc.vector.tensor_scalar_mul(
                        wu_r[:, :, 0:512:511], b3_r[:, :, 0:256:255], 4.0
                    )
                    wup3s[v] = wu

                # L1 band outputs can stream out now (split per half to
                # overlap the DMA with the rest of the image's compute)
                for band in range(3):
                    dst_r = y_d[b, band].rearrange("(u p) w -> p u w", u=4)
                    st_r = stgL1[band][:].rearrange("p (u w) -> p u w", u=4)
                    nc.sync.dma_start(dst_r[:, 0:2], st_r[:, 0:2])
                    nc.sync.dma_start(dst_r[:, 2:4], st_r[:, 2:4])
                return wup3s

            # H-upsample evacuation rotation: 2/3 ACT, 1/3 DVE
            # (GPSIMD cannot read PSUM).
            def evac(dst_ap, src_ap, k):
                if k % 3 == 2:
                    nc.vector.tensor_copy(dst_ap, src_ap)
                else:
                    nc.scalar.copy(dst_ap, src_ap)

            def stage_b1(b, wup3s, sts):
                """H-up blocks 0 and 3 + halo row-swap DMAs for image b."""
                w0, w1 = wup3s
                k = 0
                for j, (Uw, wsrc) in ((0, (U0, w0)), (3, (U3, w1))):
                    for band in range(3):
                        if j == 0:
                            st = stg2pool.tile([P, 2048], F16, tag=f"s2b{band}")
                            sts.append(st)
                        else:
                            st = sts[band]
                        up = psUp.tile([P, 512], F32, tag="up")
                        nc.tensor.matmul(
                            up[:], Uw, wsrc[:, 512 * band : 512 * (band + 1)],
                            start=True, stop=True,
                        )
                        evac(st[:, 512 * j : 512 * j + 512], up[:], k)
                        k += 1
                # halo row swap (same partition index, cross tile)
                nc.sync.dma_start(w0[0:1, :], w1[0:1, :])
                nc.sync.dma_start(w1[127:128, :], w0[127:128, :])
                # blocks 0 and 3 of the upsampled bands can stream out now
                for band in range(3):
                    dst_r = y_d[b, 3 + band].rearrange("(u p) w -> p u w", u=4)
                    st_r = sts[band][:].rearrange("p (u w) -> p u w", u=4)
                    nc.sync.dma_start(dst_r[:, 0:4:3], st_r[:, 0:4:3])

            def stage_b2(b, wup3s, sts):
                """H-up blocks 1 and 2 (halo'd rhs) + output DMA for image b."""
                w0, w1 = wup3s
                k = 3
                for j, (Uw, wsrc) in ((1, (U1p, w0)), (2, (U2p, w1))):
                    for band in range(3):
                        st = sts[band]
                        up = psUp.tile([P, 512], F32, tag="up")
                        nc.tensor.matmul(
                            up[:], Uw, wsrc[:, 512 * band : 512 * (band + 1)],
                            start=True, stop=True,
                        )
                        evac(st[:, 512 * j : 512 * j + 512], up[:], k)
                        k += 1
                for band in range(3):
                    dst_r = y_d[b, 3 + band].rearrange("(u p) w -> p u w", u=4)
                    st_r = sts[band][:].rearrange("p (u w) -> p u w", u=4)
                    nc.sync.dma_start(dst_r[:, 1:3], st_r[:, 1:3])

            pending = None
            for b in range(IMG):
                if pending is not None:
                    stage_b1(pending[0], pending[1], pending[2])
                wup3s = stage_a(b)
                if pending is not None:
                    stage_b2(pending[0], pending[1], pending[2])
                pending = (b, wup3s, [])
            stage_b1(pending[0], pending[1], pending[2])
            stage_b2(pending[0], pending[1], pending[2])

    nc.compile()
    return nc


_NC_CACHE = None
LAST_RESULTS = None


def kernel(**inputs) -> np.ndarray:
    global _NC_CACHE, LAST_RESULTS
    trace = bool(inputs.pop("_trace", False))
    x = np.asarray(inputs["x"])
    assert x.shape == (B, 1, H, W), x.shape
    x16 = np.ascontiguousarray(x.astype(np.float16))
    if _NC_CACHE is None:
        _NC_CACHE = build_nc()
    nc = _NC_CACHE
    w16 = _build_w16()
    in_maps = [
        {"xc": np.ascontiguousarray(x16[IMG * c : IMG * (c + 1), 0]), "w16": w16}
        for c in range(NCORES)
    ]
    res = bass_utils.run_bass_kernel_spmd(
        nc, in_maps, core_ids=list(range(NCORES)), trace=trace
    )
    LAST_RESULTS = res
    out = np.concatenate([res.results[c]["yc"] for c in range(NCORES)], axis=0)
    return out.astype(np.float32)


if __name__ == "__main__":
    rng = np.random.default_rng(0)
    x = rng.standard_normal((B, 1, H, W), dtype=np.float32)
    y = kernel(x=x)
    print("kernel output:", y.shape, y.dtype)


# revision 27
# speedup vs baseline: 1.1134x; 1.1134x over previous
"""Trainium2 Bass kernel for nn_DWTExtractor: 2-level Haar DWT + bilinear 2x upsample.

Input  x: (32, 1, 1024, 1024) fp32
Output y: (32, 6, 512, 512) fp32 = [cH1, cV1, cD1, cH2u, cV2u, cD2u]

Sharding: pure batch data-parallel, 4 images per core across 8 cores.

v3: fp16 datapath to halve HBM traffic (the kernel is memory-bound).
  - Host converts x to fp16; device reads fp16, writes fp16 outputs; host
    converts back to fp32. Max rel err ~6e-4 (validated in sim_v2.py).
  - L1/L2 Haar row-pairing as fp16 matmuls (+-0.5 weights), PSUM fp32 held
    as [P,1024]/[P,512] tiles.
  - Engines may read at most ONE operand from PSUM, so ACT makes one strided
    fp32 copy of the even W-columns per PSUM tile; DVE/GPSIMD then combine
    it with the strided odd columns read directly from PSUM:
      cH1 = Se - So (DVE), cA1 = Se + So (GPSIMD, fp16 for L2 rhs),
      cV1 = De + Do, cD1 = De - Do (DVE); same pattern for the L2 bands.
  - L2 uses explicit cA1 chunks -> 2 matmuls per tile.
  - W-upsample folds the x3 into scalar_tensor_tensor (split DVE/GPSIMD).
  - H-upsample in fp16, block-major with a "halo" row swap between the two
    wu tiles (SBUF->SBUF DMA) so the cross-block U1b/U2b correction matmuls
    disappear: 12 matmuls instead of 18 per image.
  - stage_b is split so the halo DMA latency hides behind the next image's
    L1 matmuls; H-up PSUM evacuation is spread over ACT/DVE/GPSIMD.
"""

import numpy as np

import concourse.bass as bass
import concourse.tile as tile
import concourse.mybir as mybir
from concourse import bacc, bass_utils

F32 = mybir.dt.float32
F16 = mybir.dt.float16
AL = mybir.AluOpType

B, H, W = 32, 1024, 1024
NCORES = 8
IMG = B // NCORES  # images per core
HL, WL = H // 2, W // 2  # 512, 512 (level-1 band size)
H2, W2 = H // 4, W // 4  # 256, 256 (level-2 band size)
P = 128


def _build_w16() -> np.ndarray:
    """(128, 8*128) fp16: PS_lo|PS_hi|PD_lo|PD_hi|U0|U1p|U2p|U3.

    U1p/U2p carry the cross-block bilinear taps in rows 0/127; the matching
    rhs rows are swapped between the wu tiles at runtime (halo DMA).
    """
    ps_lo = np.zeros((P, P), np.float16)
    ps_hi = np.zeros((P, P), np.float16)
    pd_lo = np.zeros((P, P), np.float16)
    pd_hi = np.zeros((P, P), np.float16)
    for i in range(64):
        ps_lo[2 * i, i] = 0.5
        ps_lo[2 * i + 1, i] = 0.5
        ps_hi[2 * i, 64 + i] = 0.5
        ps_hi[2 * i + 1, 64 + i] = 0.5
        pd_lo[2 * i, i] = 0.5
        pd_lo[2 * i + 1, i] = -0.5
        pd_hi[2 * i, 64 + i] = 0.5
        pd_hi[2 * i + 1, 64 + i] = -0.5

    u_full = np.zeros((H2, HL), np.float32)
    for m in range(HL):
        k = m // 2
        taps = [(k, 0.75), (k - 1, 0.25)] if m % 2 == 0 else [(k, 0.75), (k + 1, 0.25)]
        for src, wgt in taps:
            u_full[min(max(src, 0), H2 - 1), m] += wgt
    u_full *= 0.25
    u0 = u_full[0:128, 0:128].astype(np.float16)
    u1p = u_full[0:128, 128:256].astype(np.float16)
    u1p[0, :] = u_full[128, 128:256].astype(np.float16)  # halo tap row
    u2p = u_full[128:256, 256:384].astype(np.float16)
    u2p[127, :] = u_full[127, 256:384].astype(np.float16)  # halo tap row
    u3 = u_full[128:256, 384:512].astype(np.float16)

    return np.concatenate([ps_lo, ps_hi, pd_lo, pd_hi, u0, u1p, u2p, u3], axis=1)


def build_nc() -> "bacc.Bacc":
    nc = bacc.Bacc(
        "TRN2", target_bir_lowering=False, debug=False, num_devices=NCORES,
        name="dwt_extractor",
    )
    x_d = nc.dram_tensor("xc", [IMG, H, W], F16, kind="ExternalInput")
    w16_d = nc.dram_tensor("w16", [P, 8 * P], F16, kind="ExternalInput")
    y_d = nc.dram_tensor("yc", [IMG, 6, HL, WL], F16, kind="ExternalOutput")

    with tile.TileContext(nc) as tc:
        with (
            tc.tile_pool(name="consts", bufs=1) as cpool,
            tc.tile_pool(name="xin", bufs=6) as xpool,
            tc.tile_pool(name="ev", bufs=3) as evpool,
            tc.tile_pool(name="ca", bufs=6) as capool,
            tc.tile_pool(name="stg", bufs=2) as stgpool,
            tc.tile_pool(name="b3", bufs=3) as b3pool,
            tc.tile_pool(name="wup", bufs=4) as wuppool,
            tc.tile_pool(name="stg2", bufs=2) as stg2pool,
            tc.tile_pool(name="psS", bufs=1, space="PSUM") as psS,
            tc.tile_pool(name="psD", bufs=1, space="PSUM") as psD,
            tc.tile_pool(name="psL2", bufs=1, space="PSUM") as psL2,
            tc.tile_pool(name="psUp", bufs=2, space="PSUM") as psUp,
        ):
            w16 = cpool.tile([P, 8 * P], F16)
            nc.sync.dma_start(w16[:], w16_d[:])
            blk = lambda i: w16[:, i * P : (i + 1) * P]
            PS_lo, PS_hi, PD_lo, PD_hi = blk(0), blk(1), blk(2), blk(3)
            U0, U1p, U2p, U3 = blk(4), blk(5), blk(6), blk(7)

            def stage_a(b, part, ca, stgL1):
                """L1 chunks (part 0: chunks 0-1, part 1: chunks 2-3 + L2 +
                W-upsample) for image b; part 1 returns wup3s."""
                for u in (0, 1) if part == 0 else (2, 3):
                    xu = xpool.tile([P, 2048], F16, tag="x")
                    src = x_d[b, 256 * u : 256 * (u + 1), :]
                    nc.sync.dma_start(
                        xu[:].rearrange("p (t w) -> p t w", t=2),
                        src.rearrange("(t p) w -> p t w", t=2),
                    )
                    if u == 0:
                        for nm in ("sH1", "sV1", "sD1"):
                            stgL1.append(
                                stgpool.tile([P, 2048], F16, tag=nm, name=nm)
                            )
                    o512 = 512 * u
                    sS = psS.tile([P, 1024], F32, tag="S")
                    for h in range(2):
                        o = 512 * h
                        nc.tensor.matmul(
                            sS[:, o : o + 512], PS_lo, xu[:, o : o + 512],
                            start=True, stop=False,
                        )
                        nc.tensor.matmul(
                            sS[:, o : o + 512], PS_hi, xu[:, 1024 + o : 1536 + o],
                            start=False, stop=True,
                        )
                    se = evpool.tile([P, 512], F16, tag="se")
                    nc.scalar.copy(se[:], sS[:, 0:1024:2])
                    cu = capool.tile([P, 512], F16, tag="ca")
                    nc.vector.tensor_tensor(
                        stgL1[0][:, o512 : o512 + 512],
                        se[:], sS[:, 1:1024:2], AL.subtract,
                    )  # cH1
                    nc.vector.tensor_tensor(
                        cu[:], se[:], sS[:, 1:1024:2], AL.add
                    )  # cA1
                    ca.append(cu)

                    sD = psD.tile([P, 1024], F32, tag="D")
                    for h in range(2):
                        o = 512 * h
                        nc.tensor.matmul(
                            sD[:, o : o + 512], PD_lo, xu[:, o : o + 512],
                            start=True, stop=False,
                        )
                        nc.tensor.matmul(
                            sD[:, o : o + 512], PD_hi, xu[:, 1024 + o : 1536 + o],
                            start=False, stop=True,
                        )
                    de = evpool.tile([P, 512], F16, tag="de")
                    nc.scalar.copy(de[:], sD[:, 0:1024:2])
                    nc.vector.tensor_tensor(
                        stgL1[1][:, o512 : o512 + 512],
                        de[:], sD[:, 1:1024:2], AL.add,
                    )  # cV1
                    nc.vector.tensor_tensor(
                        stgL1[2][:, o512 : o512 + 512],
                        de[:], sD[:, 1:1024:2], AL.subtract,
                    )  # cD1

                if part == 0:
                    return None

                # level 2 + W-upsample; wup3s[v] = (128, 3*512) fp16
                wup3s = [None, None]
                for v in range(2):
                    s2 = psL2.tile([P, 512], F32, tag="s2")
                    nc.tensor.matmul(s2[:], PS_lo, ca[2 * v][:], start=True, stop=False)
                    nc.tensor.matmul(s2[:], PS_hi, ca[2 * v + 1][:], start=False, stop=True)
                    d2 = psL2.tile([P, 512], F32, tag="d2")
                    nc.tensor.matmul(d2[:], PD_lo, ca[2 * v][:], start=True, stop=False)
                    nc.tensor.matmul(d2[:], PD_hi, ca[2 * v + 1][:], start=False, stop=True)

                    s2e = evpool.tile([P, 256], F16, tag="s2e")
                    nc.scalar.copy(s2e[:], s2[:, 0:512:2])
                    d2e = evpool.tile([P, 256], F16, tag="d2e")
                    nc.scalar.copy(d2e[:], d2[:, 0:512:2])

                    b3 = b3pool.tile([P, 768], F16, tag="b3")
                    nc.vector.tensor_tensor(
                        b3[:, 0:256], s2e[:], s2[:, 1:512:2], AL.subtract
                    )  # cH2
                    nc.vector.tensor_tensor(
                        b3[:, 256:512], d2e[:], d2[:, 1:512:2], AL.add
                    )  # cV2
                    nc.vector.tensor_tensor(
                        b3[:, 512:768], d2e[:], d2[:, 1:512:2], AL.subtract
                    )  # cD2

                    # W-upsample: wu[2j] = 3*b[j] + b[j-1]; wu[2j+1] = 3*b[j] + b[j+1]
                    # GPSIMD has no scalar_tensor_tensor (and its fp16
                    # tensor_scalar is a very slow emulation), so stage
                    # t3 = 3*b3 on DVE and keep the adds on GPSIMD.
                    t3 = b3pool.tile([P, 768], F16, tag="t3")
                    nc.vector.tensor_scalar_mul(t3[:], b3[:], 3.0)
                    wu = wuppool.tile([P, 1536], F16, tag="wup")
                    wu_r = wu[:].rearrange("p (b w) -> p b w", b=3)
                    b3_r = b3[:].rearrange("p (b w) -> p b w", b=3)
                    t3_r = t3[:].rearrange("p (b w) -> p b w", b=3)
                    nc.gpsimd.tensor_tensor(
                        wu_r[:, :, 2:512:2], t3_r[:, :, 1:256],
                        b3_r[:, :, 0:255], AL.add,
                    )
                    nc.gpsimd.tensor_tensor(
                        wu_r[:, :, 1:511:2], t3_r[:, :, 0:255],
                        b3_r[:, :, 1:256], AL.add,
                    )
                    nc.vector.tensor_scalar_mul(
                        wu_r[:, :, 0:512:511], b3_r[:, :, 0:256:255], 4.0
                    )
                    wup3s[v] = wu

                # L1 band outputs can stream out now (GPSIMD-triggered so the
                # Sync queue stays dedicated to input prefetch)
                for band in range(3):
                    dst = y_d[b, band]
                    nc.gpsimd.dma_start(
                        dst.rearrange("(u p) w -> p u w", u=4),
                        stgL1[band][:].rearrange("p (u w) -> p u w", u=4),
                    )
                return wup3s

            # H-upsample evacuation rotation: 2/3 ACT, 1/3 DVE
            # (GPSIMD cannot read PSUM).
            def evac(dst_ap, src_ap, k):
                if k % 3 == 2:
                    nc.vector.tensor_copy(dst_ap, src_ap)
                else:
                    nc.scalar.copy(dst_ap, src_ap)

            def stage_b1(b, wup3s, sts):
                """H-up blocks 0 and 3 + halo row-swap DMAs for image b."""
                w0, w1 = wup3s
                k = 0
                for j, (Uw, wsrc) in ((0, (U0, w0)), (3, (U3, w1))):
                    for band in range(3):
                        if j == 0:
                            st = stg2pool.tile([P, 2048], F16, tag=f"s2b{band}")
                            sts.append(st)
                        else:
                            st = sts[band]
                        up = psUp.tile([P, 512], F32, tag="up")
                        nc.tensor.matmul(
                            up[:], Uw, wsrc[:, 512 * band : 512 * (band + 1)],
                            start=True, stop=True,
                        )
                        evac(st[:, 512 * j : 512 * j + 512], up[:], k)
                        k += 1
                # halo row swap (same partition index, cross tile)
                nc.sync.dma_start(w0[0:1, :], w1[0:1, :])
                nc.sync.dma_start(w1[127:128, :], w0[127:128, :])

            def stage_b2(b, wup3s, sts):
                """H-up blocks 1 and 2 (halo'd rhs) + output DMA for image b."""
                w0, w1 = wup3s
                k = 3
                for j, (Uw, wsrc) in ((1, (U1p, w0)), (2, (U2p, w1))):
                    for band in range(3):
                        st = sts[band]
                        up = psUp.tile([P, 512], F32, tag="up")
                        nc.tensor.matmul(
                            up[:], Uw, wsrc[:, 512 * band : 512 * (band + 1)],
                            start=True, stop=True,
                        )
                        evac(st[:, 512 * j : 512 * j + 512], up[:], k)
                        k += 1
                for band in range(3):
                    dst = y_d[b, 3 + band]
                    nc.gpsimd.dma_start(
                        dst.rearrange("(u p) w -> p u w", u=4),
                        sts[band][:].rearrange("p (u w) -> p u w", u=4),
                    )

            # Pipeline: the previous image's H-upsample halves are issued
            # INSIDE the next image's stage_a so PE never head-of-line blocks
            # on the wup chain or the halo DMA.
            pending = None
            for b in range(IMG):
                ca, stgL1 = [], []
                stage_a(b, 0, ca, stgL1)
                if pending is not None:
                    stage_b1(pending[0], pending[1], pending[2])
                wup3s = stage_a(b, 1, ca, stgL1)
                if pending is not None:
                    stage_b2(pending[0], pending[1], pending[2])
                pending = (b, wup3s, [])
            stage_b1(pending[0], pending[1], pending[2])
            stage_b2(pending[0], pending[1], pending[2])

    nc.compile()
    return nc


_NC_CACHE = None
LAST_RESULTS = None


def kernel(**inputs) -> np.ndarray:
    global _NC_CACHE, LAST_RESULTS
    trace = bool(inputs.pop("_trace", False))
    x = np.asarray(inputs["x"])
    assert x.shape == (B, 1, H, W), x.shape
    x16 = np.ascontiguousarray(x.astype(np.float16))
    if _NC_CACHE is None:
        _NC_CACHE = build_nc()
    nc = _NC_CACHE
    w16 = _build_w16()
    in_maps = [
        {"xc": np.ascontiguousarray(x16[IMG * c : IMG * (c + 1), 0]), "w16": w16}
        for c in range(NCORES)
    ]
    res = bass_utils.run_bass_kernel_spmd(
        nc, in_maps, core_ids=list(range(NCORES)), trace=trace
    )
    LAST_RESULTS = res
    out = np.concatenate([res.results[c]["yc"] for c in range(NCORES)], axis=0)
    return out.astype(np.float32)


if __name__ == "__main__":
    rng = np.random.default_rng(0)
    x = rng.standard_normal((B, 1, H, W), dtype=np.float32)
    y = kernel(x=x)
    print("kernel output:", y.shape, y.dtype)


# revision 28
# speedup vs baseline: 1.1735x; 1.0539x over previous
"""Trainium2 Bass kernel for nn_DWTExtractor: 2-level Haar DWT + bilinear 2x upsample.

Input  x: (32, 1, 1024, 1024) fp32
Output y: (32, 6, 512, 512) fp32 = [cH1, cV1, cD1, cH2u, cV2u, cD2u]

Sharding: pure batch data-parallel, 4 images per core across 8 cores.

v3: fp16 datapath to halve HBM traffic (the kernel is memory-bound).
  - Host converts x to fp16; device reads fp16, writes fp16 outputs; host
    converts back to fp32. Max rel err ~6e-4 (validated in sim_v2.py).
  - L1/L2 Haar row-pairing as fp16 matmuls (+-0.5 weights), PSUM fp32 held
    as [P,1024]/[P,512] tiles.
  - Engines may read at most ONE operand from PSUM, so ACT makes one strided
    fp32 copy of the even W-columns per PSUM tile; DVE/GPSIMD then combine
    it with the strided odd columns read directly from PSUM:
      cH1 = Se - So (DVE), cA1 = Se + So (GPSIMD, fp16 for L2 rhs),
      cV1 = De + Do, cD1 = De - Do (DVE); same pattern for the L2 bands.
  - L2 uses explicit cA1 chunks -> 2 matmuls per tile.
  - W-upsample folds the x3 into scalar_tensor_tensor (split DVE/GPSIMD).
  - H-upsample in fp16, block-major with a "halo" row swap between the two
    wu tiles (SBUF->SBUF DMA) so the cross-block U1b/U2b correction matmuls
    disappear: 12 matmuls instead of 18 per image.
  - stage_b is split so the halo DMA latency hides behind the next image's
    L1 matmuls; H-up PSUM evacuation is spread over ACT/DVE/GPSIMD.
"""

import numpy as np

import concourse.bass as bass
import concourse.tile as tile
import concourse.mybir as mybir
from concourse import bacc, bass_utils

F32 = mybir.dt.float32
F16 = mybir.dt.float16
AL = mybir.AluOpType

B, H, W = 32, 1024, 1024
NCORES = 8
IMG = B // NCORES  # images per core
HL, WL = H // 2, W // 2  # 512, 512 (level-1 band size)
H2, W2 = H // 4, W // 4  # 256, 256 (level-2 band size)
P = 128


def _build_w16() -> np.ndarray:
    """(128, 8*128) fp16: PS_lo|PS_hi|PD_lo|PD_hi|U0|U1p|U2p|U3.

    U1p/U2p carry the cross-block bilinear taps in rows 0/127; the matching
    rhs rows are swapped between the wu tiles at runtime (halo DMA).
    """
    ps_lo = np.zeros((P, P), np.float16)
    ps_hi = np.zeros((P, P), np.float16)
    pd_lo = np.zeros((P, P), np.float16)
    pd_hi = np.zeros((P, P), np.float16)
    for i in range(64):
        ps_lo[2 * i, i] = 0.5
        ps_lo[2 * i + 1, i] = 0.5
        ps_hi[2 * i, 64 + i] = 0.5
        ps_hi[2 * i + 1, 64 + i] = 0.5
        pd_lo[2 * i, i] = 0.5
        pd_lo[2 * i + 1, i] = -0.5
        pd_hi[2 * i, 64 + i] = 0.5
        pd_hi[2 * i + 1, 64 + i] = -0.5

    u_full = np.zeros((H2, HL), np.float32)
    for m in range(HL):
        k = m // 2
        taps = [(k, 0.75), (k - 1, 0.25)] if m % 2 == 0 else [(k, 0.75), (k + 1, 0.25)]
        for src, wgt in taps:
            u_full[min(max(src, 0), H2 - 1), m] += wgt
    u_full *= 0.25
    u0 = u_full[0:128, 0:128].astype(np.float16)
    u1p = u_full[0:128, 128:256].astype(np.float16)
    u1p[0, :] = u_full[128, 128:256].astype(np.float16)  # halo tap row
    u2p = u_full[128:256, 256:384].astype(np.float16)
    u2p[127, :] = u_full[127, 256:384].astype(np.float16)  # halo tap row
    u3 = u_full[128:256, 384:512].astype(np.float16)

    return np.concatenate([ps_lo, ps_hi, pd_lo, pd_hi, u0, u1p, u2p, u3], axis=1)


def build_nc() -> "bacc.Bacc":
    nc = bacc.Bacc(
        "TRN2", target_bir_lowering=False, debug=False, num_devices=NCORES,
        name="dwt_extractor",
    )
    x_d = nc.dram_tensor("xc", [IMG, H, W], F16, kind="ExternalInput")
    w16_d = nc.dram_tensor("w16", [P, 8 * P], F16, kind="ExternalInput")
    y_d = nc.dram_tensor("yc", [IMG, 6, HL, WL], F16, kind="ExternalOutput")

    with tile.TileContext(nc) as tc:
        with (
            tc.tile_pool(name="consts", bufs=1) as cpool,
            tc.tile_pool(name="xin", bufs=6) as xpool,
            tc.tile_pool(name="ev", bufs=3) as evpool,
            tc.tile_pool(name="ca", bufs=6) as capool,
            tc.tile_pool(name="stg", bufs=2) as stgpool,
            tc.tile_pool(name="b3", bufs=3) as b3pool,
            tc.tile_pool(name="wup", bufs=4) as wuppool,
            tc.tile_pool(name="stg2", bufs=2) as stg2pool,
            tc.tile_pool(name="psS", bufs=1, space="PSUM") as psS,
            tc.tile_pool(name="psD", bufs=1, space="PSUM") as psD,
            tc.tile_pool(name="psL2", bufs=1, space="PSUM") as psL2,
            tc.tile_pool(name="psUp", bufs=2, space="PSUM") as psUp,
        ):
            w16 = cpool.tile([P, 8 * P], F16)
            nc.sync.dma_start(w16[:], w16_d[:])
            blk = lambda i: w16[:, i * P : (i + 1) * P]
            PS_lo, PS_hi, PD_lo, PD_hi = blk(0), blk(1), blk(2), blk(3)
            U0, U1p, U2p, U3 = blk(4), blk(5), blk(6), blk(7)

            def stage_a(b, part, ca, stgL1):
                """L1 chunks (part 0: chunks 0-1, part 1: chunks 2-3 + L2 +
                W-upsample) for image b; part 1 returns wup3s."""
                for u in (0, 1) if part == 0 else (2, 3):
                    xu = xpool.tile([P, 2048], F16, tag="x")
                    src = x_d[b, 256 * u : 256 * (u + 1), :]
                    nc.sync.dma_start(
                        xu[:].rearrange("p (t w) -> p t w", t=2),
                        src.rearrange("(t p) w -> p t w", t=2),
                    )
                    if u == 0:
                        for nm in ("sH1", "sV1", "sD1"):
                            stgL1.append(
                                stgpool.tile([P, 2048], F16, tag=nm, name=nm)
                            )
                    o512 = 512 * u
                    sS = psS.tile([P, 1024], F32, tag="S")
                    for h in range(2):
                        o = 512 * h
                        nc.tensor.matmul(
                            sS[:, o : o + 512], PS_lo, xu[:, o : o + 512],
                            start=True, stop=False,
                        )
                        nc.tensor.matmul(
                            sS[:, o : o + 512], PS_hi, xu[:, 1024 + o : 1536 + o],
                            start=False, stop=True,
                        )
                    se = evpool.tile([P, 512], F16, tag="se")
                    nc.scalar.copy(se[:], sS[:, 0:1024:2])
                    cu = capool.tile([P, 512], F16, tag="ca")
                    nc.vector.tensor_tensor(
                        stgL1[0][:, o512 : o512 + 512],
                        se[:], sS[:, 1:1024:2], AL.subtract,
                    )  # cH1
                    nc.vector.tensor_tensor(
                        cu[:], se[:], sS[:, 1:1024:2], AL.add
                    )  # cA1
                    ca.append(cu)

                    sD = psD.tile([P, 1024], F32, tag="D")
                    for h in range(2):
                        o = 512 * h
                        nc.tensor.matmul(
                            sD[:, o : o + 512], PD_lo, xu[:, o : o + 512],
                            start=True, stop=False,
                        )
                        nc.tensor.matmul(
                            sD[:, o : o + 512], PD_hi, xu[:, 1024 + o : 1536 + o],
                            start=False, stop=True,
                        )
                    de = evpool.tile([P, 512], F16, tag="de")
                    nc.scalar.copy(de[:], sD[:, 0:1024:2])
                    nc.vector.tensor_tensor(
                        stgL1[1][:, o512 : o512 + 512],
                        de[:], sD[:, 1:1024:2], AL.add,
                    )  # cV1
                    nc.vector.tensor_tensor(
                        stgL1[2][:, o512 : o512 + 512],
                        de[:], sD[:, 1:1024:2], AL.subtract,
                    )  # cD1

                if part == 0:
                    return None

                # level 2 + W-upsample; wup3s[v] = (128, 3*512) fp16
                wup3s = [None, None]
                for v in range(2):
                    s2 = psL2.tile([P, 512], F32, tag="s2")
                    nc.tensor.matmul(s2[:], PS_lo, ca[2 * v][:], start=True, stop=False)
                    nc.tensor.matmul(s2[:], PS_hi, ca[2 * v + 1][:], start=False, stop=True)
                    d2 = psL2.tile([P, 512], F32, tag="d2")
                    nc.tensor.matmul(d2[:], PD_lo, ca[2 * v][:], start=True, stop=False)
                    nc.tensor.matmul(d2[:], PD_hi, ca[2 * v + 1][:], start=False, stop=True)

                    s2e = evpool.tile([P, 256], F16, tag="s2e")
                    nc.scalar.copy(s2e[:], s2[:, 0:512:2])
                    d2e = evpool.tile([P, 256], F16, tag="d2e")
                    nc.scalar.copy(d2e[:], d2[:, 0:512:2])

                    b3 = b3pool.tile([P, 768], F16, tag="b3")
                    nc.vector.tensor_tensor(
                        b3[:, 0:256], s2e[:], s2[:, 1:512:2], AL.subtract
                    )  # cH2
                    nc.vector.tensor_tensor(
                        b3[:, 256:512], d2e[:], d2[:, 1:512:2], AL.add
                    )  # cV2
                    nc.vector.tensor_tensor(
                        b3[:, 512:768], d2e[:], d2[:, 1:512:2], AL.subtract
                    )  # cD2

                    # W-upsample: wu[2j] = 3*b[j] + b[j-1]; wu[2j+1] = 3*b[j] + b[j+1]
                    # GPSIMD has no scalar_tensor_tensor (and its fp16
                    # tensor_scalar is a very slow emulation), so stage
                    # t3 = 3*b3 on DVE and keep the adds on GPSIMD.
                    t3 = b3pool.tile([P, 768], F16, tag="t3")
                    nc.vector.tensor_scalar_mul(t3[:], b3[:], 3.0)
                    wu = wuppool.tile([P, 1536], F16, tag="wup")
                    wu_r = wu[:].rearrange("p (b w) -> p b w", b=3)
                    b3_r = b3[:].rearrange("p (b w) -> p b w", b=3)
                    t3_r = t3[:].rearrange("p (b w) -> p b w", b=3)
                    nc.gpsimd.tensor_tensor(
                        wu_r[:, :, 2:512:2], t3_r[:, :, 1:256],
                        b3_r[:, :, 0:255], AL.add,
                    )
                    nc.gpsimd.tensor_tensor(
                        wu_r[:, :, 1:511:2], t3_r[:, :, 0:255],
                        b3_r[:, :, 1:256], AL.add,
                    )
                    nc.vector.tensor_scalar_mul(
                        wu_r[:, :, 0:512:511], b3_r[:, :, 0:256:255], 4.0
                    )
                    wup3s[v] = wu

                # L1 band outputs can stream out now (GPSIMD-triggered so the
                # Sync queue stays dedicated to input prefetch)
                for band in range(3):
                    dst = y_d[b, band]
                    nc.sync.dma_start(
                        dst.rearrange("(u p) w -> p u w", u=4),
                        stgL1[band][:].rearrange("p (u w) -> p u w", u=4),
                    )
                return wup3s

            # H-upsample evacuation rotation: 2/3 ACT, 1/3 DVE
            # (GPSIMD cannot read PSUM).
            def evac(dst_ap, src_ap, k):
                if k % 3 == 2:
                    nc.vector.tensor_copy(dst_ap, src_ap)
                else:
                    nc.scalar.copy(dst_ap, src_ap)

            def stage_b1(b, wup3s, sts):
                """H-up blocks 0 and 3 + halo row-swap DMAs for image b."""
                w0, w1 = wup3s
                k = 0
                for j, (Uw, wsrc) in ((0, (U0, w0)), (3, (U3, w1))):
                    for band in range(3):
                        if j == 0:
                            st = stg2pool.tile([P, 2048], F16, tag=f"s2b{band}")
                            sts.append(st)
                        else:
                            st = sts[band]
                        up = psUp.tile([P, 512], F32, tag="up")
                        nc.tensor.matmul(
                            up[:], Uw, wsrc[:, 512 * band : 512 * (band + 1)],
                            start=True, stop=True,
                        )
                        evac(st[:, 512 * j : 512 * j + 512], up[:], k)
                        k += 1
                # halo row swap (same partition index, cross tile)
                nc.sync.dma_start(w0[0:1, :], w1[0:1, :])
                nc.sync.dma_start(w1[127:128, :], w0[127:128, :])

            def stage_b2(b, wup3s, sts):
                """H-up blocks 1 and 2 (halo'd rhs) + output DMA for image b."""
                w0, w1 = wup3s
                k = 3
                for j, (Uw, wsrc) in ((1, (U1p, w0)), (2, (U2p, w1))):
                    for band in range(3):
                        st = sts[band]
                        up = psUp.tile([P, 512], F32, tag="up")
                        nc.tensor.matmul(
                            up[:], Uw, wsrc[:, 512 * band : 512 * (band + 1)],
                            start=True, stop=True,
                        )
                        evac(st[:, 512 * j : 512 * j + 512], up[:], k)
                        k += 1
                for band in range(3):
                    dst = y_d[b, 3 + band]
                    nc.sync.dma_start(
                        dst.rearrange("(u p) w -> p u w", u=4),
                        sts[band][:].rearrange("p (u w) -> p u w", u=4),
                    )

            # Pipeline: the previous image's H-upsample halves are issued
            # INSIDE the next image's stage_a so PE never head-of-line blocks
            # on the wup chain or the halo DMA.
            pending = None
            for b in range(IMG):
                ca, stgL1 = [], []
                stage_a(b, 0, ca, stgL1)
                if pending is not None:
                    stage_b1(pending[0], pending[1], pending[2])
                wup3s = stage_a(b, 1, ca, stgL1)
                if pending is not None:
                    stage_b2(pending[0], pending[1], pending[2])
                pending = (b, wup3s, [])
            stage_b1(pending[0], pending[1], pending[2])
            stage_b2(pending[0], pending[1], pending[2])

    nc.compile()
    return nc


_NC_CACHE = None
LAST_RESULTS = None


def kernel(**inputs) -> np.ndarray:
    global _NC_CACHE, LAST_RESULTS
    trace = bool(inputs.pop("_trace", False))
    x = np.asarray(inputs["x"])
    assert x.shape == (B, 1, H, W), x.shape
    x16 = np.ascontiguousarray(x.astype(np.float16))
    if _NC_CACHE is None:
        _NC_CACHE = build_nc()
    nc = _NC_CACHE
    w16 = _build_w16()
    in_maps = [
        {"xc": np.ascontiguousarray(x16[IMG * c : IMG * (c + 1), 0]), "w16": w16}
        for c in range(NCORES)
    ]
    res = bass_utils.run_bass_kernel_spmd(
        nc, in_maps, core_ids=list(range(NCORES)), trace=trace
    )
    LAST_RESULTS = res
    out = np.concatenate([res.results[c]["yc"] for c in range(NCORES)], axis=0)
    return out.astype(np.float32)


if __name__ == "__main__":
    rng = np.random.default_rng(0)
    x = rng.standard_normal((B, 1, H, W), dtype=np.float32)
    y = kernel(x=x)
    print("kernel output:", y.shape, y.dtype)
